# revision 49
# baseline (speedup 1.0000x reference)
"""CascadeAttention TRN2 kernel — 8-core head-sharded tensor parallel.

Sharding: each of the 8 NeuronCores owns 4 query heads + 1 KV head (GQA group).
Per core: qkv projections, RoPE, cascade attention over (sink + window +
current) keys with causal masking on the current block, softmax (no
max-subtraction; scores are small), o_proj partial product; host sums the 8
o_proj partials.

v2 design (vs the fp32r baseline):
- All big payloads bf16: same PE speed (1 cycle/row), half the DMA bytes,
  DVE 2x modes on elementwise ops. PSUM accumulation stays fp32.
- Softmax denominator via transposed tiny matmuls: stationary = 128-col chunk
  of the exp tile, moving = ones [128,1], output [128q, 1] accumulated in one
  PSUM bank across key tiles. Cost-model charge is output free size (=1) per
  matmul, so the old [1,512] den matmuls (~92us of PE) become ~free, and the
  denominator lands q-on-partitions so the reciprocal is a [128,8] op.
- V projection emitted directly in [k, hd] layout (stationary = hidden-state
  chunk, moving = Wv tile): kills the PE transposes + ACT copies of the
  baseline at identical matmul cost.
- Cache K is pre-roped on the host (pure input transform), so no device-side
  cache rope; cache K/V DMA straight into resident bf16 tiles.
- PSUM->SBUF copies off the exp critical path: DVE for V/attention-out
  tiles, ACT for o_proj outputs (ACT is idle during phase C). GpSimd/Pool
  cannot access PSUM on TRN2.
- Normalize: recip [128,8] -> PE transpose -> [8,128] -> DRAM bounce ->
  per-head broadcast rows [128,512], final mul on DVE at 2x.
"""
import os
import sys

for _p in ("/root/.axon_site/_ro/trn_rl_repo", "/opt/trn_rl_repo"):
    if os.path.isdir(_p) and _p not in sys.path:
        sys.path.insert(0, _p)


import numpy as np

import concourse.bass as bass
import concourse.mybir as mybir
import concourse.tile as tile
from concourse.bass_utils import run_bass_kernel_spmd
from concourse.vector_clock import ScopedClock, VectorClock

F32 = mybir.dt.float32
F16 = mybir.dt.float16
BF16 = mybir.dt.bfloat16
NP_BF16 = ml_dtypes.bfloat16
AF = mybir.ActivationFunctionType

B, Q, D = 1, 2048, 4096
H, KVH, HD = 32, 8, 128
NS, NW = 4, 2048
G = H // KVH           # q heads per kv head = heads per core
NC_CORES = 8
ROPE_BASE = 10000.0

QB = 512               # q block (matmul moving dim)
NQB = Q // QB          # 4
NDT = D // 128         # 32 contraction tiles
NKC = 17               # cache key tiles: 4 sink + 2048 window + 124 pad = 2176
KC = NKC * 128         # 2176
SCALE = 1.0 / float(np.sqrt(HD))
NEG = float(np.finfo(np.float32).min)
NCH_Q = QB // 128      # 4 q chunks per block


# ---------------------------------------------------------------------------
# TileContext tail-drain patch: stock _drain_and_barrier puts one sync-wait per
# outstanding processor on a single SP Drain, overflowing walrus's per-
# instruction wait slots. Split the waits across per-proc SP NoOps instead.
def _split_drain_and_barrier(self, tick_clock, wait_clock):
    nc = self.nc
    gc = tick_clock.global_clock
    n = len(gc)
    for i in range(n):
        t = gc[i]
        if t > 0:
            vec = [0] * n
            vec[i] = t
            nop = nc.sync.nop(nofuse=True, hint=f"tail_wait_p{i}")
            wait_clock.add_sem_waits(nop.ins, ScopedClock({None: VectorClock(vec)}))
    drain_inst = nc.sync.drain()
    full = ScopedClock({None: tick_clock.global_clock})
    wait_clock.add_sem_waits(drain_inst.ins, full, full.copy())
    nc.all_engine_barrier()
    assert self.sems is not None
    popped = nc._tile_sem_poison_stack.pop()
    assert popped is self._sem_poison
    nc.clear_and_free_semaphores(list(self.sems.allocated().values()))
    nc.all_engine_barrier()


tile.TileContext._drain_and_barrier = _split_drain_and_barrier


def _split_excess_waits(nc, cap=1):
    """Walrus enforces small per-instruction sync-wait limits (1-2 depending
    on the lowered encoding). Tile emits up to ~4 on body instructions and
    more on drains. Move excess waits onto same-engine NoOps placed directly
    before the instruction — sems are monotonic in the kernel body, so
    waiting earlier on the same engine is semantically identical."""
    import bass_rust as _br
    for f in nc.m.functions:
        for bb in f.blocks:
            il = bb.instructions
            out = []
            changed = False
            for inst in il:
                si = inst.sync_info
                waits = list(si.on_wait) if (si is not None and si.on_wait) else []
                if len(waits) > cap:
                    changed = True
                    for j, w in enumerate(waits[:-cap]):
                        nop = mybir.InstNoOp(
                            name=f"{inst.name}-w{j}", ins=[], outs=[])
                        nop.engine = inst.engine
                        nop.sync_info = _br.SyncInfo(on_wait=[w], on_update=[])
                        nc.register_instruction(nop, overwrite=True)
                        out.append(nop)
                    inst.sync_info = _br.SyncInfo(
                        on_wait=waits[-cap:],
                        on_update=list(si.on_update) if si.on_update else [])
                out.append(inst)
            if changed:
                il.clear()
                il.extend(out)


def _rope_math(nc, dst, pcp, cos_ap, sin_ap, t1, t2):
    """dst = pcp*cos + rot(pcp)*sin, in [hd, n] layout, all-2-byte SBUF
    operands (DVE 2x/4x modes). SBUF-SBUF DVE ops need equal input base
    partitions, so the sin table is half-swapped and sign-baked: rows 0:63
    hold +sin, rows 64:127 -sin. The PSUM->bf16 narrowing copies are done
    separately (batched first, to free the projection banks early)."""
    nc.vector.tensor_mul(t1, pcp, cos_ap)
    nc.vector.tensor_mul(t2[0:64, :], pcp[64:128, :], sin_ap[64:128, :])
    nc.vector.tensor_mul(t2[64:128, :], pcp[0:64, :], sin_ap[0:64, :])
    nc.vector.tensor_add(dst, t1, t2)


def _c_quanta(nc, aoT, wo_s, out_d, cps, obuf, qb):
    """o_proj chunk emitters for q-block qb: 8 dc x 4 qt, each 4 matmuls
    into one PSUM bank + DVE copy-out + store DMA."""
    for dc in range(D // QB):
        for qt in range(qb * NCH_Q, (qb + 1) * NCH_Q):
            def emit(dc=dc, qt=qt):
                pc = cps.tile([128, QB], F32, tag="pc")
                for ht in range(G):
                    nc.tensor.matmul(
                        pc[:], aoT[ht][:, bass.ts(qt, 128)],
                        wo_s[:, ht * D + dc * QB:ht * D + (dc + 1) * QB],
                        start=(ht == 0), stop=(ht == G - 1))
                ob = obuf.tile([128, QB], BF16, tag="ob")
                nc.vector.tensor_copy(ob[:], pc[:])
                nc.sync.dma_start(
                    out=out_d[qt * 128:(qt + 1) * 128,
                              dc * QB:(dc + 1) * QB],
                    in_=ob[:])
            yield emit


def _phase_b(nc, tc, qT, kcurT, v_s, ktc_r, vc_s, maskb_s, causal_s,
             onec_s, identf_s, aoT, wo_s, out_d):
    """Attention: heads processed in 2 groups of 2; exp batched per group
    ([128, 2*QB] per ACT instruction, one PSUM 2-bank sc tile per kt).
    Denominator: per kt, up to 8 single-shot transposed tiny matmuls (ex chunk
    stationary, ones moving, start=stop) into a rotating [128,8] PSUM tile,
    DVE-accumulated into an SBUF den_acc. (Interleaved multi-write PSUM
    accumulation groups within one bank are broken on TRN2; single-shot
    writes to disjoint regions are fine.)"""
    NG = 2  # heads per group
    NCH = QB // 128  # 4 q chunks per block
    with tc.tile_pool(name="ex", bufs=4) as expool, \
         tc.tile_pool(name="nrm", bufs=3) as nrm, \
         tc.tile_pool(name="drs", bufs=2, space="DRAM") as drs, \
         tc.tile_pool(name="ob", bufs=4) as obuf, \
         tc.tile_pool(name="scps", bufs=2, space="PSUM") as scps, \
         tc.tile_pool(name="avps", bufs=1, space="PSUM") as avps, \
         tc.tile_pool(name="dnps", bufs=1, space="PSUM") as dnps, \
         tc.tile_pool(name="cps", bufs=1, space="PSUM") as cps:
        for qb in range(NQB):
            cols = bass.ts(qb, QB)
            nkt = NKC + G * qb + G
            # o_proj chunks of the previous q-block, interleaved into this
            # block's kt loop to fill the ACT-paced PE gaps.
            aux = (_c_quanta(nc, aoT, wo_s, out_d, cps, obuf, qb - 1)
                   if qb >= 1 else None)
            aux_n, aux_done, it = (D // QB) * NCH_Q, 0, 0
            HOLD = 5  # kts before the first aux (aoT[qb-1] muls settling)
            iters_total = 2 * nkt
            for grp in range(G // NG):
                heads = [grp * NG + i for i in range(NG)]
                # den_acc[q, i*4+c] accumulates softmax denominators in SBUF.
                den_acc = nrm.tile([128, NG * NCH], F32, tag="den_acc")
                po = [avps.tile([128, QB], F32, tag=f"po{i}", name=f"po{i}")
                      for i in range(NG)]

                def kt_params(kt):
                    cur = kt >= NKC
                    c = kt - NKC
                    off = max(0, c * 128 - qb * QB) if cur else 0
                    diag = cur and c >= qb * (QB // 128)
                    if cur:
                        lv = v_s[:, bass.ts(c, 128)]
                        lk = kcurT[:, bass.ts(c, 128)]
                        bias = 0.0
                    else:
                        lk = ktc_r[:, bass.ts(kt, 128)]
                        lv = vc_s[:, bass.ts(kt, 128)]
                        bias = maskb_s[:, kt:kt + 1]
                    return lk, lv, bias, off, diag

                def emit_av_den(pend, start, stop):
                    ex, lv, off, kt = pend
                    for i in range(NG):
                        nc.tensor.matmul(
                            po[i][:, off:QB], lv,
                            ex[:, i * QB + off:(i + 1) * QB],
                            start=start, stop=stop)
                    c0 = off // 128  # first valid q chunk this kt
                    dp = dnps.tile([128, 8 + 128], F32, tag="dp")
                    last_dp[0] = dp
                    for i in range(NG):
                        for c in range(c0, NCH):
                            nc.tensor.matmul(
                                dp[:, i * NCH + c:i * NCH + c + 1],
                                ex[:, i * QB + c * 128:i * QB + (c + 1) * 128],
                                onec_s[:],
                                start=True, stop=True,
                                skip_group_check=True)
                    # den_acc[:, valid cols] (+)= dp[:, valid cols]
                    acc3 = den_acc[:].rearrange(
                        "p (i c) -> p i c", i=NG)[:, :, c0:NCH]
                    dp3 = dp[:, 0:NG * NCH].rearrange(
                        "p (i c) -> p i c", i=NG)[:, :, c0:NCH]
                    if kt == 0:
                        nc.vector.tensor_copy(acc3, dp3)
                    else:
                        nc.vector.tensor_add(acc3, acc3, dp3)

                # software-pipelined at depth 2: scores+exp for kt are
                # emitted two iterations before attnV/den, so the exp sem is
                # long settled when PE reaches the attnV matmul.
                pends = []  # of (ex, lv, off, kt)
                last_dp = [None]
                for kt in range(nkt):
                    lk, lv, bias, off, diag = kt_params(kt)
                    sc = scps.tile([128, NG * QB], F32, tag="sc")
                    for i, h in enumerate(heads):
                        nc.tensor.matmul(
                            sc[:, i * QB + off:(i + 1) * QB], lk,
                            qT[h][:, qb * QB + off:(qb + 1) * QB])
                    ex = expool.tile([128, NG * QB], BF16, tag="ex")
                    nc.scalar.activation(
                        ex[:].rearrange("p (g c) -> p g c", g=NG)[:, :, off:QB],
                        sc[:].rearrange("p (g c) -> p g c", g=NG)[:, :, off:QB],
                        AF.Exp, bias=bias, scale=SCALE)
                    if diag:
                        for i in range(NG):
                            nc.vector.tensor_mul(
                                ex[:, i * QB + off:i * QB + off + 128],
                                ex[:, i * QB + off:i * QB + off + 128],
                                causal_s[:])
                    pends.append((ex, lv, off, kt))
                    if len(pends) > 2:
                        p = pends.pop(0)
                        emit_av_den(p, start=(p[3] == 0), stop=False)
                    it += 1
                    if aux is not None and it > HOLD:
                        target = min(aux_n, aux_n * (it - HOLD)
                                     // (iters_total - HOLD - 2))
                        while aux_done < target:
                            next(aux)()
                            aux_done += 1
                for j, p in enumerate(pends):
                    emit_av_den(p, start=(p[3] == 0),
                                stop=(j == len(pends) - 1))

                # normalize. po banks free via Pool copies; reciprocals get
                # q-on-partitions -> PE transpose -> DRAM bounce -> per-head
                # broadcast rows -> DVE mul (all-bf16, 2x).
                po_sb = []
                for i in range(NG):
                    p_sb = nrm.tile([128, QB], BF16, tag=f"posb{i}",
                                    name=f"posb{i}")
                    nc.vector.tensor_copy(p_sb[:], po[i][:])
                    po_sb.append(p_sb)
                rec = nrm.tile([128, 8], F32, tag="rec")
                nc.vector.reciprocal(rec[:], den_acc[:])
                dpt = last_dp[0]
                nc.tensor.transpose(dpt[0:8, 8:136], rec[:], identf_s[:])
                rec_row = nrm.tile([8, 128], BF16, tag="recrow")
                nc.vector.tensor_copy(rec_row[:], dpt[0:8, 8:136])
                rdr = drs.tile([8, 128], BF16, tag="rdr")
                nc.sync.dma_start(out=rdr[:], in_=rec_row[:])
                for i, h in enumerate(heads):
                    rb = nrm.tile([128, QB], BF16, tag=f"rb{i}")
                    sl = rdr[i * NCH:(i + 1) * NCH, :]
                    bc = bass.AP(tensor=sl.tensor, offset=sl.offset,
                                 ap=[[0, 128]] + list(sl.ap))
                    nc.sync.dma_start(
                        out=rb[:].rearrange("p (c j) -> p c j", c=NCH), in_=bc)
                    nc.vector.tensor_mul(aoT[h][:, cols], po_sb[i][:], rb[:])
            if aux is not None:
                while aux_done < aux_n:
                    next(aux)()
                    aux_done += 1


DBG = bool(int(os.environ.get("CASCADE_DBG", "0")))


def build_nc():
    nc = bass.Bass()

    # ---- DRAM I/O (per-core shards) ----
    hsT_d = nc.dram_tensor("hsT", [D, Q], BF16, kind="ExternalInput")
    wq_d = nc.dram_tensor("wq", [D, G * HD], BF16, kind="ExternalInput")
    wkv_d = nc.dram_tensor("wkv", [D, 2 * HD], BF16, kind="ExternalInput")
    wo_d = nc.dram_tensor("wo", [G * HD, D], BF16, kind="ExternalInput")
    # cache K^T pre-roped on host, [hd, k] tiled; cache V packed [k%128, t*128+hd]
    ktc_d = nc.dram_tensor("ktc", [HD, KC], BF16, kind="ExternalInput")
    vc_d = nc.dram_tensor("vc", [128, KC], BF16, kind="ExternalInput")
    cosq_d = nc.dram_tensor("cosq", [HD, Q], F16, kind="ExternalInput")
    sinq_d = nc.dram_tensor("sinq", [HD, Q], F16, kind="ExternalInput")
    maskb_d = nc.dram_tensor("maskb", [128, NKC], F32, kind="ExternalInput")
    causal_d = nc.dram_tensor("causal01", [128, 128], BF16, kind="ExternalInput")
    onec_d = nc.dram_tensor("onec", [128, 1], BF16, kind="ExternalInput")
    identf_d = nc.dram_tensor("identf", [128, 128], F32, kind="ExternalInput")
    out_d = nc.dram_tensor("out", [Q, D], BF16, kind="ExternalOutput")
    if DBG:
        dbgq_d = nc.dram_tensor("dbgq", [G * 128, Q], BF16, kind="ExternalOutput")
        dbgk_d = nc.dram_tensor("dbgk", [128, Q], BF16, kind="ExternalOutput")
        dbgv_d = nc.dram_tensor("dbgv", [128, Q], BF16, kind="ExternalOutput")
        dbgao_d = nc.dram_tensor("dbgao", [G * 128, Q], BF16, kind="ExternalOutput")

    with tile.TileContext(nc) as tc:
        # ---------------- resident tiles (live across phases) --------------
        with tc.tile_pool(name="res", bufs=1) as res, \
             tc.tile_pool(name="small", bufs=1) as small:
            qT = [res.tile([128, Q], BF16, tag=f"qT{h}", name=f"qT{h}")
                  for h in range(G)]
            kcurT = res.tile([128, Q], BF16, tag="kcurT")
            v_s = res.tile([128, Q], BF16, tag="v_s")       # [k%128, c*128+hd]
            ktc_r = res.tile([128, KC], BF16, tag="ktc_r")  # roped cache K^T
            vc_s = res.tile([128, KC], BF16, tag="vc_s")    # cache V tiles
            aoT = [res.tile([128, Q], BF16, tag=f"aoT{h}", name=f"aoT{h}")
                   for h in range(G)]
            wo_s = res.tile([128, G * D], BF16, tag="wo_s")  # [hd, ht*D+oc]
            maskb_s = small.tile([128, NKC], F32, tag="maskb")
            causal_s = small.tile([128, 128], BF16, tag="causal")
            onec_s = small.tile([128, 1], BF16, tag="onec")
            identf_s = small.tile([128, 128], F32, tag="identf")


            # ---------------- phase A: projections + rope ------------------
            with tc.tile_pool(name="wqkv", bufs=1) as wpool, \
                 tc.tile_pool(name="hst", bufs=1) as hpool, \
                 tc.tile_pool(name="hst2", bufs=2) as hpool2, \
                 tc.tile_pool(name="tabq", bufs=2) as tabq, \
                 tc.tile_pool(name="scr", bufs=2) as scr, \
                 tc.tile_pool(name="pjps", bufs=1, space="PSUM") as pjps:
                NCH = 4
                DCH = NDT // NCH  # d-tiles per wq chunk
                wq_s = [wpool.tile([128, DCH * G * HD], BF16, tag=f"wq{i}",
                                   name=f"wq{i}") for i in range(NCH)]
                wkv_s = wpool.tile([128, NDT * 2 * HD], BF16, tag="wkv")

                def dma_wq(i, half=None):
                    lo = 0 if half in (None, 0) else DCH // 2
                    hi = DCH if half in (None, 1) else DCH // 2
                    nc.sync.dma_start(
                        out=wq_s[i][:].rearrange(
                            "p (t c) -> p t c", t=DCH)[:, lo:hi],
                        in_=wq_d[(i * DCH + lo) * 128:(i * DCH + hi) * 128, :]
                            .rearrange("(t p) c -> p t c", p=128),
                    )

                # DMA order tuned for startup: first wq half + first hst
                # tiles feed the first matmuls; wkv arrives by pk(dt0);
                # wq1-3 stream during the qb0 loop; the big B/C payloads
                # (cache K/V, Wo, tables) are emitted at the end of qb0.
                dma_wq(0, half=0)

                for qb in range(NQB):
                    cols = bass.ts(qb, QB)
                    pq = [pjps.tile([128, QB], F32, tag=f"pq{h}",
                                    name=f"pq{h}") for h in range(G)]
                    pk = pjps.tile([128, QB], F32, tag="pk")
                    pv = pjps.tile([128, QB], F32, tag="pv")
                    # hidden-state tiles resident for the whole qb block (the
                    # pv sweeps below re-read them per chunk).
                    hst = []
                    for dt in range(NDT):
                        # first few d-tiles double-buffered: their next-block
                        # DMAs would otherwise wait on this block's last pv
                        # sweep and stall the next block's first matmuls
                        pool_ = hpool2 if dt < 4 else hpool
                        ht_ = pool_.tile([128, QB], BF16, tag=f"hst{dt}",
                                         name=f"hst{dt}")
                        nc.sync.dma_start(
                            out=ht_, in_=hsT_d[dt * 128:(dt + 1) * 128, cols])
                        hst.append(ht_)
                        if qb == 0:
                            if dt == 1:
                                dma_wq(0, half=1)
                            elif dt == 3:
                                nc.sync.dma_start(
                                    out=wkv_s[:].rearrange(
                                        "p (t c) -> p t c", t=NDT),
                                    in_=wkv_d[:, :].rearrange(
                                        "(t p) c -> p t c", p=128),
                                )
                            elif dt in (8, 14, 20):
                                dma_wq({8: 1, 14: 2, 20: 3}[dt])
                    cosq_s = tabq.tile([128, QB], F16, tag="cosq")
                    sinq_s = tabq.tile([128, QB], F16, tag="sinq")
                    nc.sync.dma_start(out=cosq_s, in_=cosq_d[:, cols])
                    nc.sync.dma_start(out=sinq_s, in_=sinq_d[:, cols])
                    def emit_qk():
                        for dt in range(NDT):
                            st = dict(start=(dt == 0), stop=(dt == NDT - 1))
                            wqc = wq_s[dt // DCH]
                            dto = dt % DCH
                            for h in range(G):
                                nc.tensor.matmul(
                                    pq[h][:],
                                    wqc[:, dto * G * HD + h * HD:
                                        dto * G * HD + (h + 1) * HD],
                                    hst[dt][:], **st,
                                )
                            nc.tensor.matmul(
                                pk[:], wkv_s[:, dt * 2 * HD:dt * 2 * HD + HD],
                                hst[dt][:], **st)

                    def emit_v():
                        # V direct to [k, hd]: stationary = hst q-chunk,
                        # moving = Wv tile -> pv[:, c*128:...] = [128q,
                        # 128hd]. Per-chunk accumulation is contiguous
                        # (interleaved PSUM groups in one bank are broken
                        # on TRN2).
                        for c in range(NCH):
                            for dt in range(NDT):
                                nc.tensor.matmul(
                                    pv[:, bass.ts(c, 128)],
                                    hst[dt][:, bass.ts(c, 128)],
                                    wkv_s[:, dt * 2 * HD + HD:
                                          (dt + 1) * 2 * HD],
                                    start=(dt == 0), stop=(dt == NDT - 1),
                                    skip_group_check=True)

                    emit_qk()
                    emit_v()
                    # rope q heads + current k into resident bf16 tiles.
                    # All five PSUM->bf16 narrowing copies go first so the
                    # projection banks free as fast as DVE can drain them.
                    pcps = []
                    for j, src_ps in enumerate(pq + [pk]):
                        pcp = scr.tile([128, QB], BF16, tag=f"pcp{j}",
                                       name=f"pcp{j}")
                        nc.vector.tensor_copy(pcp[:], src_ps[:])
                        pcps.append(pcp)
                    # current V: DVE copies pv chunks into v_s (before the
                    # rope math so the pv bank hands off to phase B fast)
                    for c in range(NCH):
                        nc.vector.tensor_copy(
                            v_s[:, bass.ts(qb * NCH + c, 128)],
                            pv[:, bass.ts(c, 128)])
                    for j, dst in enumerate(
                            [qT[h][:, cols] for h in range(G)]
                            + [kcurT[:, cols]]):
                        t1 = scr.tile([128, QB], BF16, tag="t1")
                        t2 = scr.tile([128, QB], BF16, tag="t2")
                        _rope_math(nc, dst, pcps[j][:],
                                   cosq_s[:], sinq_s[:], t1[:], t2)
                    if qb == 0:
                        # big B/C payloads + small tables, off the startup
                        # critical path
                        nc.sync.dma_start(out=maskb_s, in_=maskb_d[:, :])
                        nc.sync.dma_start(out=causal_s, in_=causal_d[:, :])
                        nc.sync.dma_start(out=onec_s, in_=onec_d[:, :])
                        nc.sync.dma_start(out=identf_s, in_=identf_d[:, :])
                        nc.sync.dma_start(out=ktc_r, in_=ktc_d[:, :])
                        nc.sync.dma_start(out=vc_s, in_=vc_d[:, :])
                        nc.sync.dma_start(
                            out=wo_s[:].rearrange("p (t c) -> p t c", t=G),
                            in_=wo_d[:, :].rearrange("(t p) c -> p t c", p=128),
                        )

            # -------- phase B: attention, with o_proj chunks interleaved ---
            _phase_b(nc, tc, qT, kcurT, v_s, ktc_r, vc_s, maskb_s,
                     causal_s, onec_s, identf_s, aoT, wo_s, out_d)
            # last q-block's o_proj: B pools are closed, so it gets deep
            # PSUM double-buffering of its own.
            with tc.tile_pool(name="ob2", bufs=4) as obuf2, \
                 tc.tile_pool(name="cps2", bufs=4, space="PSUM") as cps2:
                for emit in _c_quanta(nc, aoT, wo_s, out_d, cps2, obuf2,
                                      NQB - 1):
                    emit()
            if DBG:
                for h in range(G):
                    nc.sync.dma_start(
                        out=dbgq_d[h * 128:(h + 1) * 128, :], in_=qT[h][:])
                    nc.sync.dma_start(
                        out=dbgao_d[h * 128:(h + 1) * 128, :], in_=aoT[h][:])
                nc.sync.dma_start(out=dbgk_d[:, :], in_=kcurT[:])
                nc.sync.dma_start(out=dbgv_d[:, :], in_=v_s[:])

    _split_excess_waits(nc)
    return nc


_NC_CACHE = None


def _get_nc():
    global _NC_CACHE
    if _NC_CACHE is None:
        _NC_CACHE = build_nc()
    return _NC_CACHE


def _tables(pos):
    """cos/sin tables in [hd, n] layout; sin rows 0:63 negated (rope rot)."""
    inv_freq = 1.0 / (ROPE_BASE ** (np.arange(0, HD, 2, dtype=np.float32)
                                    / np.float32(HD)))
    inv_freq = inv_freq.astype(np.float32)
    ang = (pos.astype(np.float32)[None, :]
           * inv_freq[:, None]).astype(np.float32)
    a64 = ang.astype(np.float64)
    cos = np.cos(a64).astype(np.float32)
    sin = np.sin(a64).astype(np.float32)
    cosT = np.concatenate([cos, cos], axis=0)
    sinT = np.concatenate([sin, -sin], axis=0)  # half-swapped (see _rope_tiles)
    return (np.ascontiguousarray(cosT).astype(np.float16),
            np.ascontiguousarray(sinT).astype(np.float16))


def _rope_host(x, pos):
    """x [n, HD] -> roped, matching reference _rope (f64 angles)."""
    inv_freq = 1.0 / (ROPE_BASE ** (np.arange(0, HD, 2, dtype=np.float32)
                                    .astype(np.float32) / np.float32(HD)))
    ang = (pos.astype(np.float32)[:, None]
           * inv_freq.astype(np.float32)[None, :]).astype(np.float64)
    cos = np.cos(ang)
    sin = np.sin(ang)
    x = x.astype(np.float64)
    x1, x2 = x[:, :HD // 2], x[:, HD // 2:]
    return np.concatenate([x1 * cos - x2 * sin, x2 * cos + x1 * sin],
                          axis=1).astype(np.float32)


def _prepare_in_maps(hidden_states, sink_k, sink_v, win_k, win_v, sink_pos,
                     key_pos, sink_mask, key_mask, Wq, Wk, Wv, Wo):
    hs = np.asarray(hidden_states, dtype=np.float32)[0]        # [Q, D]
    hsT = np.ascontiguousarray(hs.T).astype(NP_BF16)            # [D, Q]
    Wq = np.asarray(Wq, dtype=np.float32)
    Wk = np.asarray(Wk, dtype=np.float32)
    Wv = np.asarray(Wv, dtype=np.float32)
    Wo = np.asarray(Wo, dtype=np.float32)
    sink_k = np.asarray(sink_k, dtype=np.float32)
    sink_v = np.asarray(sink_v, dtype=np.float32)
    win_k = np.asarray(win_k, dtype=np.float32)
    win_v = np.asarray(win_v, dtype=np.float32)
    spos = np.asarray(sink_pos).astype(np.int64)
    kpos = np.asarray(key_pos).astype(np.int64)
    smask = np.asarray(sink_mask, dtype=np.float32)
    kmask = np.asarray(key_mask, dtype=np.float32)

    max_pos = max(int(spos.max()), int(kpos.max())) + 1
    qpos = np.arange(Q, dtype=np.float64) + max_pos
    cosq, sinq = _tables(qpos)                                  # [128, Q]

    maskb = np.concatenate(
        [smask, kmask, np.ones(KC - NS - NW, np.float32)]).astype(np.float32)
    maskb = maskb * np.float32(NEG)
    maskb_T = np.ascontiguousarray(maskb.reshape(NKC, 128).T)   # [128, NKC]

    causal01 = (np.arange(128)[:, None] <= np.arange(128)[None, :]) \
        .astype(NP_BF16)                                        # keep k<=q
    onec = np.ones((128, 1), NP_BF16)
    identf = np.eye(128, dtype=np.float32)

    Wq_h = Wq.reshape(D, H, HD)
    Wo_h = Wo.reshape(H, HD, D)
    pad = KC - NS - NW

    in_maps = []
    for c in range(NC_CORES):
        hsel = slice(c * G, (c + 1) * G)
        wq_c = np.ascontiguousarray(
            Wq_h[:, hsel].reshape(D, G * HD)).astype(NP_BF16)
        wkv_c = np.ascontiguousarray(np.concatenate(
            [Wk[:, c * HD:(c + 1) * HD], Wv[:, c * HD:(c + 1) * HD]],
            axis=1)).astype(NP_BF16)
        wo_c = np.ascontiguousarray(
            Wo_h[hsel].reshape(G * HD, D)).astype(NP_BF16)
        # cache K: host-roped, [KC, HD] -> [HD, KC]
        kc = np.concatenate([_rope_host(sink_k[0, c], spos),
                             _rope_host(win_k[0, c], kpos),
                             np.zeros((pad, HD), np.float32)], axis=0)
        ktc = np.ascontiguousarray(kc.T).astype(NP_BF16)        # [HD, KC]
        # cache V packed: vc[p, t*128+j] = V[t*128+p, j]
        vcat = np.concatenate([sink_v[0, c], win_v[0, c],
                               np.zeros((pad, HD), np.float32)], axis=0)
        vp = np.ascontiguousarray(
            vcat.reshape(NKC, 128, HD).transpose(1, 0, 2)
            .reshape(128, NKC * HD)).astype(NP_BF16)
        in_maps.append(dict(
            hsT=hsT, wq=wq_c, wkv=wkv_c, wo=wo_c,
            ktc=ktc, vc=vp,
            cosq=cosq, sinq=sinq,
            maskb=maskb_T, causal01=causal01,
            onec=onec, identf=identf,
        ))

    return in_maps


def kernel(**inputs):
    in_maps = _prepare_in_maps(**inputs)
    nc = _get_nc()
    res = run_bass_kernel_spmd(nc, in_maps, list(range(NC_CORES)))
    acc = np.zeros((Q, D), dtype=np.float64)
    for r in res.results:
        acc += r["out"].astype(np.float64)
    return acc.astype(np.float32)[None]


if __name__ == "__main__":
    nc = build_nc()
    ni = sum(len(bb.instructions) for f in nc.m.functions for bb in f.blocks)
    print(f"built ok: {ni} instructions")


# revision 50
# speedup vs baseline: 1.0048x; 1.0048x over previous
"""CascadeAttention TRN2 kernel — 8-core head-sharded tensor parallel.

Sharding: each of the 8 NeuronCores owns 4 query heads + 1 KV head (GQA group).
Per core: qkv projections, RoPE, cascade attention over (sink + window +
current) keys with causal masking on the current block, softmax (no
max-subtraction; scores are small), o_proj partial product; host sums the 8
o_proj partials.

v2 design (vs the fp32r baseline):
- All big payloads bf16: same PE speed (1 cycle/row), half the DMA bytes,
  DVE 2x modes on elementwise ops. PSUM accumulation stays fp32.
- Softmax denominator via transposed tiny matmuls: stationary = 128-col chunk
  of the exp tile, moving = ones [128,1], output [128q, 1] accumulated in one
  PSUM bank across key tiles. Cost-model charge is output free size (=1) per
  matmul, so the old [1,512] den matmuls (~92us of PE) become ~free, and the
  denominator lands q-on-partitions so the reciprocal is a [128,8] op.
- V projection emitted directly in [k, hd] layout (stationary = hidden-state
  chunk, moving = Wv tile): kills the PE transposes + ACT copies of the
  baseline at identical matmul cost.
- Cache K is pre-roped on the host (pure input transform), so no device-side
  cache rope; cache K/V DMA straight into resident bf16 tiles.
- PSUM->SBUF copies off the exp critical path: DVE for V/attention-out
  tiles, ACT for o_proj outputs (ACT is idle during phase C). GpSimd/Pool
  cannot access PSUM on TRN2.
- Normalize: recip [128,8] -> PE transpose -> [8,128] -> DRAM bounce ->
  per-head broadcast rows [128,512], final mul on DVE at 2x.
"""
import os
import sys

for _p in ("/root/.axon_site/_ro/trn_rl_repo", "/opt/trn_rl_repo"):
    if os.path.isdir(_p) and _p not in sys.path:
        sys.path.insert(0, _p)


import numpy as np

import concourse.bass as bass
import concourse.mybir as mybir
import concourse.tile as tile
from concourse.bass_utils import run_bass_kernel_spmd
from concourse.vector_clock import ScopedClock, VectorClock

F32 = mybir.dt.float32
F16 = mybir.dt.float16
BF16 = mybir.dt.bfloat16
NP_BF16 = ml_dtypes.bfloat16
AF = mybir.ActivationFunctionType

B, Q, D = 1, 2048, 4096
H, KVH, HD = 32, 8, 128
NS, NW = 4, 2048
G = H // KVH           # q heads per kv head = heads per core
NC_CORES = 8
ROPE_BASE = 10000.0

QB = 512               # q block (matmul moving dim)
NQB = Q // QB          # 4
NDT = D // 128         # 32 contraction tiles
NKC = 17               # cache key tiles: 4 sink + 2048 window + 124 pad = 2176
KC = NKC * 128         # 2176
SCALE = 1.0 / float(np.sqrt(HD))
NEG = float(np.finfo(np.float32).min)
NCH_Q = QB // 128      # 4 q chunks per block


# ---------------------------------------------------------------------------
# TileContext tail-drain patch: stock _drain_and_barrier puts one sync-wait per
# outstanding processor on a single SP Drain, overflowing walrus's per-
# instruction wait slots. Split the waits across per-proc SP NoOps instead.
def _split_drain_and_barrier(self, tick_clock, wait_clock):
    nc = self.nc
    gc = tick_clock.global_clock
    n = len(gc)
    for i in range(n):
        t = gc[i]
        if t > 0:
            vec = [0] * n
            vec[i] = t
            nop = nc.sync.nop(nofuse=True, hint=f"tail_wait_p{i}")
            wait_clock.add_sem_waits(nop.ins, ScopedClock({None: VectorClock(vec)}))
    drain_inst = nc.sync.drain()
    full = ScopedClock({None: tick_clock.global_clock})
    wait_clock.add_sem_waits(drain_inst.ins, full, full.copy())
    nc.all_engine_barrier()
    assert self.sems is not None
    popped = nc._tile_sem_poison_stack.pop()
    assert popped is self._sem_poison
    nc.clear_and_free_semaphores(list(self.sems.allocated().values()))
    nc.all_engine_barrier()


tile.TileContext._drain_and_barrier = _split_drain_and_barrier


def _split_excess_waits(nc, cap=1):
    """Walrus enforces small per-instruction sync-wait limits (1-2 depending
    on the lowered encoding). Tile emits up to ~4 on body instructions and
    more on drains. Move excess waits onto same-engine NoOps placed directly
    before the instruction — sems are monotonic in the kernel body, so
    waiting earlier on the same engine is semantically identical."""
    import bass_rust as _br
    for f in nc.m.functions:
        for bb in f.blocks:
            il = bb.instructions
            out = []
            changed = False
            for inst in il:
                si = inst.sync_info
                waits = list(si.on_wait) if (si is not None and si.on_wait) else []
                if len(waits) > cap:
                    changed = True
                    for j, w in enumerate(waits[:-cap]):
                        nop = mybir.InstNoOp(
                            name=f"{inst.name}-w{j}", ins=[], outs=[])
                        nop.engine = inst.engine
                        nop.sync_info = _br.SyncInfo(on_wait=[w], on_update=[])
                        nc.register_instruction(nop, overwrite=True)
                        out.append(nop)
                    inst.sync_info = _br.SyncInfo(
                        on_wait=waits[-cap:],
                        on_update=list(si.on_update) if si.on_update else [])
                out.append(inst)
            if changed:
                il.clear()
                il.extend(out)


def _rope_math(nc, dst, pcp, cos_ap, sin_ap, t1, t2):
    """dst = pcp*cos + rot(pcp)*sin, in [hd, n] layout, all-2-byte SBUF
    operands (DVE 2x/4x modes). SBUF-SBUF DVE ops need equal input base
    partitions, so the sin table is half-swapped and sign-baked: rows 0:63
    hold +sin, rows 64:127 -sin. The PSUM->bf16 narrowing copies are done
    separately (batched first, to free the projection banks early)."""
    nc.vector.tensor_mul(t1, pcp, cos_ap)
    nc.vector.tensor_mul(t2[0:64, :], pcp[64:128, :], sin_ap[64:128, :])
    nc.vector.tensor_mul(t2[64:128, :], pcp[0:64, :], sin_ap[0:64, :])
    nc.vector.tensor_add(dst, t1, t2)


def _c_quanta(nc, aoT, wo_s, out_d, cps, obuf, qb):
    """o_proj chunk emitters for q-block qb: 8 dc x 4 qt, each 4 matmuls
    into one PSUM bank + DVE copy-out + store DMA."""
    for dc in range(D // QB):
        for qt in range(qb * NCH_Q, (qb + 1) * NCH_Q):
            def emit(dc=dc, qt=qt):
                pc = cps.tile([128, QB], F32, tag="pc")
                for ht in range(G):
                    nc.tensor.matmul(
                        pc[:], aoT[ht][:, bass.ts(qt, 128)],
                        wo_s[:, ht * D + dc * QB:ht * D + (dc + 1) * QB],
                        start=(ht == 0), stop=(ht == G - 1))
                ob = obuf.tile([128, QB], BF16, tag="ob")
                nc.vector.tensor_copy(ob[:], pc[:])
                nc.sync.dma_start(
                    out=out_d[qt * 128:(qt + 1) * 128,
                              dc * QB:(dc + 1) * QB],
                    in_=ob[:])
            yield emit


def _phase_b(nc, tc, qT, kcurT, v_s, ktc_r, vc_s, maskb_s, causal_s,
             onec_s, oner_s, identf_s, aoT, wo_s, out_d):
    """Attention: heads processed in 2 groups of 2; exp batched per group
    ([128, 2*QB] per ACT instruction, one PSUM 2-bank sc tile per kt).
    Denominator: per kt, up to 8 single-shot transposed tiny matmuls (ex chunk
    stationary, ones moving, start=stop) into a rotating [128,8] PSUM tile,
    DVE-accumulated into an SBUF den_acc. (Interleaved multi-write PSUM
    accumulation groups within one bank are broken on TRN2; single-shot
    writes to disjoint regions are fine.)"""
    NG = 2  # heads per group
    NCH = QB // 128  # 4 q chunks per block
    with tc.tile_pool(name="ex", bufs=4) as expool, \
         tc.tile_pool(name="nrm", bufs=3) as nrm, \
         tc.tile_pool(name="ob", bufs=4) as obuf, \
         tc.tile_pool(name="scps", bufs=2, space="PSUM") as scps, \
         tc.tile_pool(name="avps", bufs=1, space="PSUM") as avps, \
         tc.tile_pool(name="dnps", bufs=1, space="PSUM") as dnps, \
         tc.tile_pool(name="cps", bufs=1, space="PSUM") as cps:
        for qb in range(NQB):
            cols = bass.ts(qb, QB)
            nkt = NKC + G * qb + G
            # o_proj chunks of the previous q-block, interleaved into this
            # block's kt loop to fill the ACT-paced PE gaps.
            aux = (_c_quanta(nc, aoT, wo_s, out_d, cps, obuf, qb - 1)
                   if qb >= 1 else None)
            aux_n, aux_done, it = (D // QB) * NCH_Q, 0, 0
            HOLD = 5  # kts before the first aux (aoT[qb-1] muls settling)
            iters_total = 2 * nkt
            for grp in range(G // NG):
                heads = [grp * NG + i for i in range(NG)]
                # den_acc[q, i*4+c] accumulates softmax denominators in SBUF.
                den_acc = nrm.tile([128, NG * NCH], F32, tag="den_acc")
                po = [avps.tile([128, QB], F32, tag=f"po{i}", name=f"po{i}")
                      for i in range(NG)]

                def kt_params(kt):
                    cur = kt >= NKC
                    c = kt - NKC
                    off = max(0, c * 128 - qb * QB) if cur else 0
                    diag = cur and c >= qb * (QB // 128)
                    if cur:
                        lv = v_s[:, bass.ts(c, 128)]
                        lk = kcurT[:, bass.ts(c, 128)]
                        bias = 0.0
                    else:
                        lk = ktc_r[:, bass.ts(kt, 128)]
                        lv = vc_s[:, bass.ts(kt, 128)]
                        bias = maskb_s[:, kt:kt + 1]
                    return lk, lv, bias, off, diag

                def emit_av_den(pend, start, stop):
                    ex, lv, off, kt = pend
                    for i in range(NG):
                        nc.tensor.matmul(
                            po[i][:, off:QB], lv,
                            ex[:, i * QB + off:(i + 1) * QB],
                            start=start, stop=stop)
                    c0 = off // 128  # first valid q chunk this kt
                    dp = dnps.tile([128, 8 + 128], F32, tag="dp")
                    last_dp[0] = dp
                    for i in range(NG):
                        for c in range(c0, NCH):
                            nc.tensor.matmul(
                                dp[:, i * NCH + c:i * NCH + c + 1],
                                ex[:, i * QB + c * 128:i * QB + (c + 1) * 128],
                                onec_s[:],
                                start=True, stop=True,
                                skip_group_check=True)
                    # den_acc[:, valid cols] (+)= dp[:, valid cols]
                    acc3 = den_acc[:].rearrange(
                        "p (i c) -> p i c", i=NG)[:, :, c0:NCH]
                    dp3 = dp[:, 0:NG * NCH].rearrange(
                        "p (i c) -> p i c", i=NG)[:, :, c0:NCH]
                    if kt == 0:
                        nc.vector.tensor_copy(acc3, dp3)
                    else:
                        nc.vector.tensor_add(acc3, acc3, dp3)

                # software-pipelined at depth 2: scores+exp for kt are
                # emitted two iterations before attnV/den, so the exp sem is
                # long settled when PE reaches the attnV matmul.
                pends = []  # of (ex, lv, off, kt)
                last_dp = [None]
                for kt in range(nkt):
                    lk, lv, bias, off, diag = kt_params(kt)
                    sc = scps.tile([128, NG * QB], F32, tag="sc")
                    for i, h in enumerate(heads):
                        nc.tensor.matmul(
                            sc[:, i * QB + off:(i + 1) * QB], lk,
                            qT[h][:, qb * QB + off:(qb + 1) * QB])
                    ex = expool.tile([128, NG * QB], BF16, tag="ex")
                    nc.scalar.activation(
                        ex[:].rearrange("p (g c) -> p g c", g=NG)[:, :, off:QB],
                        sc[:].rearrange("p (g c) -> p g c", g=NG)[:, :, off:QB],
                        AF.Exp, bias=bias, scale=SCALE)
                    if diag:
                        for i in range(NG):
                            nc.vector.tensor_mul(
                                ex[:, i * QB + off:i * QB + off + 128],
                                ex[:, i * QB + off:i * QB + off + 128],
                                causal_s[:])
                    pends.append((ex, lv, off, kt))
                    if len(pends) > 2:
                        p = pends.pop(0)
                        emit_av_den(p, start=(p[3] == 0), stop=False)
                    it += 1
                    if aux is not None and it > HOLD:
                        target = min(aux_n, aux_n * (it - HOLD)
                                     // (iters_total - HOLD - 2))
                        while aux_done < target:
                            next(aux)()
                            aux_done += 1
                for j, p in enumerate(pends):
                    emit_av_den(p, start=(p[3] == 0),
                                stop=(j == len(pends) - 1))

                # normalize. po banks free via Pool copies; reciprocals get
                # q-on-partitions -> PE transpose -> DRAM bounce -> per-head
                # broadcast rows -> DVE mul (all-bf16, 2x).
                po_sb = []
                for i in range(NG):
                    p_sb = nrm.tile([128, QB], BF16, tag=f"posb{i}",
                                    name=f"posb{i}")
                    nc.vector.tensor_copy(p_sb[:], po[i][:])
                    po_sb.append(p_sb)
                rec = nrm.tile([128, 8], F32, tag="rec")
                nc.vector.reciprocal(rec[:], den_acc[:])
                dpt = last_dp[0]
                nc.tensor.transpose(dpt[0:8, 8:136], rec[:], identf_s[:])
                rec_row = nrm.tile([8, 128], BF16, tag="recrow")
                nc.vector.tensor_copy(rec_row[:], dpt[0:8, 8:136])
                # broadcast reciprocals along partitions via PE outer product
                # (ones-column x recip-row), into the just-freed po banks —
                # much shorter latency than the old DRAM bounce.
                for i, h in enumerate(heads):
                    rb_ps = avps.tile([128, QB], F32, tag=f"po{i}",
                                      name=f"rb{i}")
                    for c in range(NCH):
                        t = i * NCH + c
                        nc.tensor.matmul(
                            rb_ps[:, c * 128:(c + 1) * 128], oner_s[:],
                            rec_row[t:t + 1, :],
                            start=True, stop=True, skip_group_check=True,
                            tile_position=(0, 0))
                    nc.vector.tensor_mul(aoT[h][:, cols], po_sb[i][:],
                                         rb_ps[:])
            if aux is not None:
                while aux_done < aux_n:
                    next(aux)()
                    aux_done += 1


DBG = bool(int(os.environ.get("CASCADE_DBG", "0")))


def build_nc():
    nc = bass.Bass()

    # ---- DRAM I/O (per-core shards) ----
    hsT_d = nc.dram_tensor("hsT", [D, Q], BF16, kind="ExternalInput")
    wq_d = nc.dram_tensor("wq", [D, G * HD], BF16, kind="ExternalInput")
    wkv_d = nc.dram_tensor("wkv", [D, 2 * HD], BF16, kind="ExternalInput")
    wo_d = nc.dram_tensor("wo", [G * HD, D], BF16, kind="ExternalInput")
    # cache K^T pre-roped on host, [hd, k] tiled; cache V packed [k%128, t*128+hd]
    ktc_d = nc.dram_tensor("ktc", [HD, KC], BF16, kind="ExternalInput")
    vc_d = nc.dram_tensor("vc", [128, KC], BF16, kind="ExternalInput")
    cosq_d = nc.dram_tensor("cosq", [HD, Q], F16, kind="ExternalInput")
    sinq_d = nc.dram_tensor("sinq", [HD, Q], F16, kind="ExternalInput")
    maskb_d = nc.dram_tensor("maskb", [128, NKC], F32, kind="ExternalInput")
    causal_d = nc.dram_tensor("causal01", [128, 128], BF16, kind="ExternalInput")
    onec_d = nc.dram_tensor("onec", [128, 1], BF16, kind="ExternalInput")
    identf_d = nc.dram_tensor("identf", [128, 128], F32, kind="ExternalInput")
    out_d = nc.dram_tensor("out", [Q, D], BF16, kind="ExternalOutput")
    if DBG:
        dbgq_d = nc.dram_tensor("dbgq", [G * 128, Q], BF16, kind="ExternalOutput")
        dbgk_d = nc.dram_tensor("dbgk", [128, Q], BF16, kind="ExternalOutput")
        dbgv_d = nc.dram_tensor("dbgv", [128, Q], BF16, kind="ExternalOutput")
        dbgao_d = nc.dram_tensor("dbgao", [G * 128, Q], BF16, kind="ExternalOutput")

    with tile.TileContext(nc) as tc:
        # ---------------- resident tiles (live across phases) --------------
        with tc.tile_pool(name="res", bufs=1) as res, \
             tc.tile_pool(name="small", bufs=1) as small:
            qT = [res.tile([128, Q], BF16, tag=f"qT{h}", name=f"qT{h}")
                  for h in range(G)]
            kcurT = res.tile([128, Q], BF16, tag="kcurT")
            v_s = res.tile([128, Q], BF16, tag="v_s")       # [k%128, c*128+hd]
            ktc_r = res.tile([128, KC], BF16, tag="ktc_r")  # roped cache K^T
            vc_s = res.tile([128, KC], BF16, tag="vc_s")    # cache V tiles
            aoT = [res.tile([128, Q], BF16, tag=f"aoT{h}", name=f"aoT{h}")
                   for h in range(G)]
            wo_s = res.tile([128, G * D], BF16, tag="wo_s")  # [hd, ht*D+oc]
            maskb_s = small.tile([128, NKC], F32, tag="maskb")
            causal_s = small.tile([128, 128], BF16, tag="causal")
            onec_s = small.tile([128, 1], BF16, tag="onec")
            identf_s = small.tile([128, 128], F32, tag="identf")


            # ---------------- phase A: projections + rope ------------------
            with tc.tile_pool(name="wqkv", bufs=1) as wpool, \
                 tc.tile_pool(name="hst", bufs=1) as hpool, \
                 tc.tile_pool(name="hst2", bufs=2) as hpool2, \
                 tc.tile_pool(name="tabq", bufs=2) as tabq, \
                 tc.tile_pool(name="scr", bufs=2) as scr, \
                 tc.tile_pool(name="pjps", bufs=1, space="PSUM") as pjps:
                NCH = 4
                DCH = NDT // NCH  # d-tiles per wq chunk
                wq_s = [wpool.tile([128, DCH * G * HD], BF16, tag=f"wq{i}",
                                   name=f"wq{i}") for i in range(NCH)]
                wkv_s = wpool.tile([128, NDT * 2 * HD], BF16, tag="wkv")

                def dma_wq(i, half=None):
                    lo = 0 if half in (None, 0) else DCH // 2
                    hi = DCH if half in (None, 1) else DCH // 2
                    nc.sync.dma_start(
                        out=wq_s[i][:].rearrange(
                            "p (t c) -> p t c", t=DCH)[:, lo:hi],
                        in_=wq_d[(i * DCH + lo) * 128:(i * DCH + hi) * 128, :]
                            .rearrange("(t p) c -> p t c", p=128),
                    )

                # DMA order tuned for startup: first wq half + first hst
                # tiles feed the first matmuls; wkv arrives by pk(dt0);
                # wq1-3 stream during the qb0 loop; the big B/C payloads
                # (cache K/V, Wo, tables) are emitted at the end of qb0.
                dma_wq(0, half=0)

                for qb in range(NQB):
                    cols = bass.ts(qb, QB)
                    pq = [pjps.tile([128, QB], F32, tag=f"pq{h}",
                                    name=f"pq{h}") for h in range(G)]
                    pk = pjps.tile([128, QB], F32, tag="pk")
                    pv = pjps.tile([128, QB], F32, tag="pv")
                    # hidden-state tiles resident for the whole qb block (the
                    # pv sweeps below re-read them per chunk).
                    hst = []
                    for dt in range(NDT):
                        # first few d-tiles double-buffered: their next-block
                        # DMAs would otherwise wait on this block's last pv
                        # sweep and stall the next block's first matmuls
                        pool_ = hpool2 if dt < 4 else hpool
                        ht_ = pool_.tile([128, QB], BF16, tag=f"hst{dt}",
                                         name=f"hst{dt}")
                        nc.sync.dma_start(
                            out=ht_, in_=hsT_d[dt * 128:(dt + 1) * 128, cols])
                        hst.append(ht_)
                        if qb == 0:
                            if dt == 1:
                                dma_wq(0, half=1)
                            elif dt == 3:
                                nc.sync.dma_start(
                                    out=wkv_s[:].rearrange(
                                        "p (t c) -> p t c", t=NDT),
                                    in_=wkv_d[:, :].rearrange(
                                        "(t p) c -> p t c", p=128),
                                )
                            elif dt in (8, 14, 20):
                                dma_wq({8: 1, 14: 2, 20: 3}[dt])
                    cosq_s = tabq.tile([128, QB], F16, tag="cosq")
                    sinq_s = tabq.tile([128, QB], F16, tag="sinq")
                    nc.sync.dma_start(out=cosq_s, in_=cosq_d[:, cols])
                    nc.sync.dma_start(out=sinq_s, in_=sinq_d[:, cols])
                    def emit_qk():
                        for dt in range(NDT):
                            st = dict(start=(dt == 0), stop=(dt == NDT - 1))
                            wqc = wq_s[dt // DCH]
                            dto = dt % DCH
                            for h in range(G):
                                nc.tensor.matmul(
                                    pq[h][:],
                                    wqc[:, dto * G * HD + h * HD:
                                        dto * G * HD + (h + 1) * HD],
                                    hst[dt][:], **st,
                                )
                            nc.tensor.matmul(
                                pk[:], wkv_s[:, dt * 2 * HD:dt * 2 * HD + HD],
                                hst[dt][:], **st)

                    def emit_v():
                        # V direct to [k, hd]: stationary = hst q-chunk,
                        # moving = Wv tile -> pv[:, c*128:...] = [128q,
                        # 128hd]. Per-chunk accumulation is contiguous
                        # (interleaved PSUM groups in one bank are broken
                        # on TRN2).
                        for c in range(NCH):
                            for dt in range(NDT):
                                nc.tensor.matmul(
                                    pv[:, bass.ts(c, 128)],
                                    hst[dt][:, bass.ts(c, 128)],
                                    wkv_s[:, dt * 2 * HD + HD:
                                          (dt + 1) * 2 * HD],
                                    start=(dt == 0), stop=(dt == NDT - 1),
                                    skip_group_check=True)

                    emit_qk()
                    emit_v()
                    # rope q heads + current k into resident bf16 tiles.
                    # All five PSUM->bf16 narrowing copies go first so the
                    # projection banks free as fast as DVE can drain them.
                    pcps = []
                    for j, src_ps in enumerate(pq + [pk]):
                        pcp = scr.tile([128, QB], BF16, tag=f"pcp{j}",
                                       name=f"pcp{j}")
                        nc.vector.tensor_copy(pcp[:], src_ps[:])
                        pcps.append(pcp)
                    # current V: DVE copies pv chunks into v_s (before the
                    # rope math so the pv bank hands off to phase B fast)
                    for c in range(NCH):
                        nc.vector.tensor_copy(
                            v_s[:, bass.ts(qb * NCH + c, 128)],
                            pv[:, bass.ts(c, 128)])
                    for j, dst in enumerate(
                            [qT[h][:, cols] for h in range(G)]
                            + [kcurT[:, cols]]):
                        t1 = scr.tile([128, QB], BF16, tag="t1")
                        t2 = scr.tile([128, QB], BF16, tag="t2")
                        _rope_math(nc, dst, pcps[j][:],
                                   cosq_s[:], sinq_s[:], t1[:], t2)
                    if qb == 0:
                        # big B/C payloads + small tables, off the startup
                        # critical path
                        nc.sync.dma_start(out=maskb_s, in_=maskb_d[:, :])
                        nc.sync.dma_start(out=causal_s, in_=causal_d[:, :])
                        nc.sync.dma_start(out=onec_s, in_=onec_d[:, :])
                        nc.sync.dma_start(out=oner_s, in_=oner_d[:, :])
                        nc.sync.dma_start(out=identf_s, in_=identf_d[:, :])
                        nc.sync.dma_start(out=ktc_r, in_=ktc_d[:, :])
                        nc.sync.dma_start(out=vc_s, in_=vc_d[:, :])
                        nc.sync.dma_start(
                            out=wo_s[:].rearrange("p (t c) -> p t c", t=G),
                            in_=wo_d[:, :].rearrange("(t p) c -> p t c", p=128),
                        )

            # -------- phase B: attention, with o_proj chunks interleaved ---
            _phase_b(nc, tc, qT, kcurT, v_s, ktc_r, vc_s, maskb_s,
                     causal_s, onec_s, oner_s, identf_s, aoT, wo_s, out_d)
            # last q-block's o_proj: B pools are closed, so it gets deep
            # PSUM double-buffering of its own.
            with tc.tile_pool(name="ob2", bufs=4) as obuf2, \
                 tc.tile_pool(name="cps2", bufs=4, space="PSUM") as cps2:
                for emit in _c_quanta(nc, aoT, wo_s, out_d, cps2, obuf2,
                                      NQB - 1):
                    emit()
            if DBG:
                for h in range(G):
                    nc.sync.dma_start(
                        out=dbgq_d[h * 128:(h + 1) * 128, :], in_=qT[h][:])
                    nc.sync.dma_start(
                        out=dbgao_d[h * 128:(h + 1) * 128, :], in_=aoT[h][:])
                nc.sync.dma_start(out=dbgk_d[:, :], in_=kcurT[:])
                nc.sync.dma_start(out=dbgv_d[:, :], in_=v_s[:])

    _split_excess_waits(nc)
    return nc


_NC_CACHE = None


def _get_nc():
    global _NC_CACHE
    if _NC_CACHE is None:
        _NC_CACHE = build_nc()
    return _NC_CACHE


def _tables(pos):
    """cos/sin tables in [hd, n] layout; sin rows 0:63 negated (rope rot)."""
    inv_freq = 1.0 / (ROPE_BASE ** (np.arange(0, HD, 2, dtype=np.float32)
                                    / np.float32(HD)))
    inv_freq = inv_freq.astype(np.float32)
    ang = (pos.astype(np.float32)[None, :]
           * inv_freq[:, None]).astype(np.float32)
    a64 = ang.astype(np.float64)
    cos = np.cos(a64).astype(np.float32)
    sin = np.sin(a64).astype(np.float32)
    cosT = np.concatenate([cos, cos], axis=0)
    sinT = np.concatenate([sin, -sin], axis=0)  # half-swapped (see _rope_tiles)
    return (np.ascontiguousarray(cosT).astype(np.float16),
            np.ascontiguousarray(sinT).astype(np.float16))


def _rope_host(x, pos):
    """x [n, HD] -> roped, matching reference _rope (f64 angles)."""
    inv_freq = 1.0 / (ROPE_BASE ** (np.arange(0, HD, 2, dtype=np.float32)
                                    .astype(np.float32) / np.float32(HD)))
    ang = (pos.astype(np.float32)[:, None]
           * inv_freq.astype(np.float32)[None, :]).astype(np.float64)
    cos = np.cos(ang)
    sin = np.sin(ang)
    x = x.astype(np.float64)
    x1, x2 = x[:, :HD // 2], x[:, HD // 2:]
    return np.concatenate([x1 * cos - x2 * sin, x2 * cos + x1 * sin],
                          axis=1).astype(np.float32)


def _prepare_in_maps(hidden_states, sink_k, sink_v, win_k, win_v, sink_pos,
                     key_pos, sink_mask, key_mask, Wq, Wk, Wv, Wo):
    hs = np.asarray(hidden_states, dtype=np.float32)[0]        # [Q, D]
    hsT = np.ascontiguousarray(hs.T).astype(NP_BF16)            # [D, Q]
    Wq = np.asarray(Wq, dtype=np.float32)
    Wk = np.asarray(Wk, dtype=np.float32)
    Wv = np.asarray(Wv, dtype=np.float32)
    Wo = np.asarray(Wo, dtype=np.float32)
    sink_k = np.asarray(sink_k, dtype=np.float32)
    sink_v = np.asarray(sink_v, dtype=np.float32)
    win_k = np.asarray(win_k, dtype=np.float32)
    win_v = np.asarray(win_v, dtype=np.float32)
    spos = np.asarray(sink_pos).astype(np.int64)
    kpos = np.asarray(key_pos).astype(np.int64)
    smask = np.asarray(sink_mask, dtype=np.float32)
    kmask = np.asarray(key_mask, dtype=np.float32)

    max_pos = max(int(spos.max()), int(kpos.max())) + 1
    qpos = np.arange(Q, dtype=np.float64) + max_pos
    cosq, sinq = _tables(qpos)                                  # [128, Q]

    maskb = np.concatenate(
        [smask, kmask, np.ones(KC - NS - NW, np.float32)]).astype(np.float32)
    maskb = maskb * np.float32(NEG)
    maskb_T = np.ascontiguousarray(maskb.reshape(NKC, 128).T)   # [128, NKC]

    causal01 = (np.arange(128)[:, None] <= np.arange(128)[None, :]) \
        .astype(NP_BF16)                                        # keep k<=q
    onec = np.ones((128, 1), NP_BF16)
    identf = np.eye(128, dtype=np.float32)

    Wq_h = Wq.reshape(D, H, HD)
    Wo_h = Wo.reshape(H, HD, D)
    pad = KC - NS - NW

    in_maps = []
    for c in range(NC_CORES):
        hsel = slice(c * G, (c + 1) * G)
        wq_c = np.ascontiguousarray(
            Wq_h[:, hsel].reshape(D, G * HD)).astype(NP_BF16)
        wkv_c = np.ascontiguousarray(np.concatenate(
            [Wk[:, c * HD:(c + 1) * HD], Wv[:, c * HD:(c + 1) * HD]],
            axis=1)).astype(NP_BF16)
        wo_c = np.ascontiguousarray(
            Wo_h[hsel].reshape(G * HD, D)).astype(NP_BF16)
        # cache K: host-roped, [KC, HD] -> [HD, KC]
        kc = np.concatenate([_rope_host(sink_k[0, c], spos),
                             _rope_host(win_k[0, c], kpos),
                             np.zeros((pad, HD), np.float32)], axis=0)
        ktc = np.ascontiguousarray(kc.T).astype(NP_BF16)        # [HD, KC]
        # cache V packed: vc[p, t*128+j] = V[t*128+p, j]
        vcat = np.concatenate([sink_v[0, c], win_v[0, c],
                               np.zeros((pad, HD), np.float32)], axis=0)
        vp = np.ascontiguousarray(
            vcat.reshape(NKC, 128, HD).transpose(1, 0, 2)
            .reshape(128, NKC * HD)).astype(NP_BF16)
        in_maps.append(dict(
            hsT=hsT, wq=wq_c, wkv=wkv_c, wo=wo_c,
            ktc=ktc, vc=vp,
            cosq=cosq, sinq=sinq,
            maskb=maskb_T, causal01=causal01,
            onec=onec, oner=oner, identf=identf,
        ))

    return in_maps


def kernel(**inputs):
    in_maps = _prepare_in_maps(**inputs)
    nc = _get_nc()
    res = run_bass_kernel_spmd(nc, in_maps, list(range(NC_CORES)))
    acc = np.zeros((Q, D), dtype=np.float64)
    for r in res.results:
        acc += r["out"].astype(np.float64)
    return acc.astype(np.float32)[None]


if __name__ == "__main__":
    nc = build_nc()
    ni = sum(len(bb.instructions) for f in nc.m.functions for bb in f.blocks)
    print(f"built ok: {ni} instructions")
